# revision 90
# baseline (speedup 1.0000x reference)
"""Trainium2 Bass kernel for the circular drift-diffusion loss (batched expm).

Reference computes  loss = -mean_b log(relu(e_{idx_b}^T expm(t_b*A) p0_b) + eps)
with A a fixed 360x360 circular advection-diffusion operator, t_b in [0,1000),
p0_b a von Mises density, over a batch of 256.

Algorithm (per core; batch sharded 32/core over 8 cores):
  * Quantize t_b = m_b*T0 + r_b with T0 = 1000/2^K, m_b < 2^K.
  * Build the propagator chain M_j = expm(2^j*T0*A) by repeated squaring.
    The prelude evaluates the Taylor of expm(T0*A) in Horner form
    G_k = I + (X/k)G_{k+1}: the +k*I terms ride the PE as accumulate-
    matmuls against host-sent scaled identities and the 1/k scales fold
    into the psum->SBUF copies, so the prelude needs no elementwise adds.
  * ALL wide matmuls run in fp32r: 1 PE row/cycle (vs 4 for fp32) when
    the moving dim >= 256.  Its rounding noise through the chain is
    ~1e-3 relative on the density, i.e. ~1e-4 on the log-loss -- two
    orders inside the 2e-2 budget -- and the Taylor tolerances are
    relaxed to match (which also shortens the chain to K=7).
  * Bits 0..K-3 of m_b apply as masked batched matvecs merged into the
    squaring matmuls (32 extra moving columns); the blends are
    arithmetic (old + msk*(new-old)) since copy_predicated can't write
    fp32r.  The top TWO bits are blend-free: the kernel emits the branch
    tree Q, MQ, M^2Q, M^3Q (M = M_{K-2}), selects all four, and the host
    picks per sample -- blends between narrow applies were pure latency.
  * Residual: Q <- Taylor_DEG_R(r_b A) Q with per-sample r folded into
    host-precomputed coefficient tables; the narrow matvec steps hide
    inside the prelude rounds.
  * p0 built on device as one [P, NCH*BL]-wide op chain on a single
    engine (min-of-squares fold, Estrin cos poly, Exp activation);
    selection via one-hot + PE column-sum.  The branch pick and the
    log/mean loss tail run on host (O(B) glue).
Scheduling notes: dummy warm-up matmuls burn the PE's 2x-slow p-state
ramp inside the initial DMA shadow; loads are split across both HWDGE
queues and the gpsimd SWDGE ring because DMA transfers serialize per
queue; elementwise ops are hand-pinned to DVE/ACT/Pool so the copies
that gate each level land on an engine that is free at that moment.
"""

import math

import numpy as np

# ---------------- static problem constants (hardcoded per contract) ----------
N = 360            # color mesh size
P = 120            # partition chunk (N = 3*P)
NCH = 3            # chunks
B = 256            # total batch
NCORES = 8
BL = B // NCORES   # per-core batch
QW = NCH * BL      # width of a full Q block
T_MAX = 1000.0
KAPPA = 400.0      # 1/SIGMA_INIT^2
EPS = 1e-5
TWO_PI = 6.283185307179586
# ln(1/(2*pi*i0e(400)))  [i0e(400) = 0.019953356281939987]
LNC = 2.076480848703078
# cos(sqrt(u)) on u in [0, pi^2] (|delta| folded to [0,pi]), power basis c0..c8
COS_COEF = [1.00000000e+00, -5.00000000e-01, 4.16666666e-02, -1.38888885e-03,
            2.48015646e-05, -2.75566515e-07, 2.08651966e-09, -1.13535474e-11,
            4.13131734e-14]

_COMPILED = {}


def _taylor_deg(x, tol, lo):
    """Smallest d with x^(d+1)/(d+1)! < tol."""
    d = lo
    term = x ** (d + 1) / math.factorial(d + 1)
    while term > tol and d < 40:
        d += 1
        term *= x / (d + 1)
    return d


def _plan(anorm):
    """Choose (k_bits, deg_p, deg_r) from ||A||_inf.  The time grid is
    T0 = T_MAX/2^k_bits; every squaring level applies one bit of the
    quantized delay.  Tolerances sit just under the fp32r rounding noise
    (~1e-3 through the chain), which the 2e-2 rel-err budget dwarfs."""
    xa = T_MAX * float(anorm)
    if xa <= 0.0:
        return 2, 4, 3
    k0 = max(2, min(16, math.ceil(math.log2(max(xa / 0.9, 2.0)))))

    def degrees(k):
        x0 = xa / (1 << k)
        # tolerances are RELATIVE error on the propagated density; the log
        # in the loss divides that by |loss|~10, so a few percent is still
        # an order of magnitude inside the 2e-2 budget.  Prelude truncation
        # is amplified ~2^(k/2) through the squarings; the residual Taylor
        # is applied once (no amplification).
        tol_p = max(min(2.5e-2 / 2 ** (k / 2), 2e-3), 5e-8)
        return _taylor_deg(x0, tol_p, 2), _taylor_deg(x0, 1e-2, 2)

    # pick k by explicit cost minimization with measured per-stage costs
    # (chain level ~3.0us, prelude step ~1.8us, taylor step ~0.3us wall)
    best = None
    for k in range(max(2, k0 - 2), min(16, k0 + 2) + 1):
        dp, dr = degrees(k)
        cost = (k - 1) * 3.0 + (dp - 1) * 1.8 + dr * 0.3
        if best is None or cost < best[0]:
            best = (cost, k, dp, dr)
    _, k, deg_p, deg_r = best
    return k, deg_p, deg_r


def _build_bass(k_bits, deg_p, deg_r):
    """Construct the Bass program (SPMD; identical on all 8 cores)."""
    import concourse.tile as tile
    from concourse import bacc, mybir

    F32 = mybir.dt.float32
    F32R = mybir.dt.float32r
    AF = mybir.ActivationFunctionType
    OP = mybir.AluOpType

    nc = bacc.Bacc("TRN2", target_bir_lowering=False, debug=False)

    CW = 2 * N                 # per-chunk block in the X|XT image
    NMSK = max(k_bits - 2, 0)
    AUXW = (deg_r + NMSK + 4 * NCH) * BL

    d_xe = nc.dram_tensor("xe", [P, NCH * CW], F32R,
                          kind="ExternalInput").ap()   # [XNc|XTc] per chunk
    ke_vals = list(range(1, deg_p - 1)) + [deg_p, (deg_p - 1) * deg_p]
    if 1 not in ke_vals:
        ke_vals = [1] + ke_vals
    ke_idx = {v: i for i, v in enumerate(ke_vals)}
    d_ke = nc.dram_tensor("ke", [P, len(ke_vals) * P], F32R,
                          kind="ExternalInput").ap()   # v*I_P per slot
    d_pv = nc.dram_tensor("pv", [P, 2 * QW], F32,
                          kind="ExternalInput").ap()   # [CM3|IREP]
    d_aux = nc.dram_tensor("aux", [P, AUXW], F32,
                           kind="ExternalInput").ap()  # [RDK|MSK|OH]
    d_out = nc.dram_tensor("sel", [1, 4 * BL], F32,
                           kind="ExternalOutput").ap()

    with tile.TileContext(nc) as tc:
        with (
            tc.tile_pool(name="const", bufs=1) as cpool,
            tc.tile_pool(name="mats", bufs=4) as mpool,
            tc.tile_pool(name="qp", bufs=2) as qpool,
            tc.tile_pool(name="vp", bufs=3) as vpool,
            tc.tile_pool(name="tp", bufs=4) as tpool,
            tc.tile_pool(name="psb", bufs=5, space="PSUM") as psb,
            tc.tile_pool(name="pss", bufs=3, space="PSUM") as pss,
        ):
            # ---- engine helpers: explicit pinning (GPSIMD can't read PSUM,
            # ACT can't do tensor_tensor; criticial-path copies go to the
            # engine that is free at that point of each level)
            def e_copy(dst, src, eng, scale=None):
                if scale is not None:
                    if eng == "dve":
                        nc.vector.tensor_scalar(dst, src, scale, None,
                                                op0=OP.mult)
                    elif eng == "act":
                        nc.scalar.mul(dst, src, scale)
                    else:
                        nc.gpsimd.tensor_scalar(dst, src, scale, None,
                                                op0=OP.mult)
                else:
                    if eng == "dve":
                        nc.vector.tensor_copy(dst, src)
                    elif eng == "act":
                        nc.scalar.copy(dst, src)
                    else:
                        nc.gpsimd.tensor_copy(dst, src)

            def e_tt(dst, a, b, op, eng):
                (nc.vector if eng == "dve" else nc.gpsimd).tensor_tensor(
                    dst, a, b, op=op)

            # ---- constants.  HWDGE costs ~625ns of serialized ring per
            # DMA, so the critical loads (pv for p0, then the X|XT chunks)
            # go there in need-order while everything else rides the
            # separate software-DGE ring (gpsimd-issued).
            # transfers serialize per hardware queue, so spread the loads
            # over all three HWDGE queues (SP/DVE/ACT): the X|XT chunks land
            # concurrently ~4.4us in instead of staggering 1us apart
            PV = cpool.tile([P, 2 * QW], F32, tag="pv")
            CXT = cpool.tile([P, NCH * CW], F32R, tag="cxt")
            nc.sync.dma_start(CXT[:, 0:CW], d_xe[:, 0:CW])
            nc.scalar.dma_start(CXT[:, CW: 2 * CW], d_xe[:, CW: 2 * CW])
            H = 2 * CW + CW // 2
            nc.gpsimd.dma_start(PV[:], d_pv[:])
            nc.sync.dma_start(CXT[:, 2 * CW: H], d_xe[:, 2 * CW: H])
            nc.scalar.dma_start(CXT[:, H: 3 * CW], d_xe[:, H: 3 * CW])
            KE = cpool.tile([P, len(ke_vals) * P], F32R, tag="ke")
            nc.gpsimd.dma_start(KE[:], d_ke[:])

            def ke_blk(v):
                return KE[:, ke_idx[v] * P: (ke_idx[v] + 1) * P]
            # fp32r identity for transposes (bf16 would rate 1.0 cyc/row vs
            # 1.5 but lowers to Ldweights+matmul pairs that stall the PE)
            E120 = KE[:, 0:P]   # ke slot 0 is 1*I
            AUX = cpool.tile([P, AUXW], F32, tag="aux")
            nc.gpsimd.dma_start(AUX[:], d_aux[:])

            # warm-up matmuls: the PE runs its first ~3us at the mid p-state
            # (2x cycle time); burning that ramp on dummies while the DMAs
            # land makes the real prelude run at full clock
            W0 = cpool.tile([P, N], F32, tag="w0m")
            nc.vector.memset(W0[:], 0.0)
            for wmw in (N, N, 250):
                wps = psb.tile([P, N], F32, tag="sq", bufs=3)
                nc.tensor.matmul(wps[:, 0:wmw], lhsT=W0[:, 0:P],
                                 rhs=W0[:, 0:wmw], start=True, stop=True)

            def xn(c):
                return CXT[:, c * CW: c * CW + N]

            def xt_blk(c, i):
                return CXT[:, c * CW + N + i * P: c * CW + N + (i + 1) * P]

            RDK = AUX[:, 0: deg_r * BL]
            MSK = AUX[:, deg_r * BL: (deg_r + NMSK) * BL]
            OH4 = AUX[:, (deg_r + NMSK) * BL: AUXW]

            ONES = cpool.tile([P, 1], F32, tag="ones")
            nc.vector.memset(ONES[:], 1.0)
            BEXP = cpool.tile([P, 1], F32, tag="bexp")
            nc.vector.memset(BEXP[:], LNC - KAPPA)

            W = N + BL  # merged chunk width: [M_c | Q_c]

            def mm_group(ps, lhsT_of, rhs_tile, i, rhs_w, rhs_stride=None):
                rs = rhs_w if rhs_stride is None else rhs_stride
                for c in range(NCH):
                    nc.tensor.matmul(
                        ps[:],
                        lhsT=lhsT_of(c, i),
                        rhs=rhs_tile[:, c * rs: c * rs + rhs_w],
                        start=(c == 0), stop=(c == NCH - 1),
                    )

            # ---- p0 (von Mises) as one [P, QW]-wide op chain.  Latency
            # matters (p0 gates the residual Taylor steps hidden inside the
            # prelude), so: u = min(d^2, (d-2pi)^2, (d+2pi)^2) needs no Abs
            # round-trip, and cos(sqrt(u)) evaluates Estrin-style (depth ~7)
            # split across DVE and Pool.
            CM3 = PV[:, 0:QW]
            IREP = PV[:, QW:2 * QW]
            Q = qpool.tile([P, QW], F32R, tag="q")
            # one engine (DVE) end to end: no cross-engine semaphore hops on
            # the latency spine, and DVE's prelude work (fused S-updates)
            # can lag since S is only read at the chain's first level
            V = nc.vector
            dl = tpool.tile([P, QW], F32, tag="w0")
            V.tensor_tensor(dl[:], IREP, CM3, op=OP.subtract)
            bm = tpool.tile([P, QW], F32, tag="w1")
            V.tensor_scalar(bm[:], dl[:], 1.0, -TWO_PI,
                            op0=OP.mult, op1=OP.add)
            cp_ = tpool.tile([P, QW], F32, tag="w2")
            V.tensor_scalar(cp_[:], dl[:], 1.0, TWO_PI,
                            op0=OP.mult, op1=OP.add)
            a2 = tpool.tile([P, QW], F32, tag="w3")
            V.tensor_tensor(a2[:], dl[:], dl[:], op=OP.mult)
            b2 = tpool.tile([P, QW], F32, tag="w1")
            V.tensor_tensor(b2[:], bm[:], bm[:], op=OP.mult)
            c2 = tpool.tile([P, QW], F32, tag="w2")
            V.tensor_tensor(c2[:], cp_[:], cp_[:], op=OP.mult)
            u = tpool.tile([P, QW], F32, tag="w0")
            V.tensor_tensor(u[:], b2[:], c2[:], op=OP.min)
            V.tensor_tensor(u[:], u[:], a2[:], op=OP.min)
            # Estrin: pairs via fused (x*s0 + s1), then combine with powers
            u2 = tpool.tile([P, QW], F32, tag="w1")
            V.tensor_tensor(u2[:], u[:], u[:], op=OP.mult)
            p01 = tpool.tile([P, QW], F32, tag="w2")
            V.tensor_scalar(p01[:], u[:], COS_COEF[1], COS_COEF[0],
                            op0=OP.mult, op1=OP.add)
            p23 = tpool.tile([P, QW], F32, tag="w3")
            V.tensor_scalar(p23[:], u[:], COS_COEF[3], COS_COEF[2],
                            op0=OP.mult, op1=OP.add)
            p45 = tpool.tile([P, QW], F32, tag="w4")
            V.tensor_scalar(p45[:], u[:], COS_COEF[5], COS_COEF[4],
                            op0=OP.mult, op1=OP.add)
            p67 = tpool.tile([P, QW], F32, tag="w5")
            V.tensor_scalar(p67[:], u[:], COS_COEF[7], COS_COEF[6],
                            op0=OP.mult, op1=OP.add)
            u4 = tpool.tile([P, QW], F32, tag="w6")
            V.tensor_tensor(u4[:], u2[:], u2[:], op=OP.mult)
            # q67 = p67 + c8*u2 ; hi = p45 + q67*u2 ; lo = p01 + p23*u2
            q67 = tpool.tile([P, QW], F32, tag="w1")
            V.tensor_scalar(q67[:], u2[:], COS_COEF[8], None, op0=OP.mult)
            V.tensor_tensor(q67[:], q67[:], p67[:], op=OP.add)
            V.tensor_tensor(q67[:], q67[:], u2[:], op=OP.mult)
            hi = tpool.tile([P, QW], F32, tag="w4")
            V.tensor_tensor(hi[:], p45[:], q67[:], op=OP.add)
            lo = tpool.tile([P, QW], F32, tag="w2")
            V.tensor_tensor(p23[:], p23[:], u2[:], op=OP.mult)
            V.tensor_tensor(lo[:], p01[:], p23[:], op=OP.add)
            h = tpool.tile([P, QW], F32, tag="w0")
            V.tensor_tensor(h[:], hi[:], u4[:], op=OP.mult)
            V.tensor_tensor(h[:], h[:], lo[:], op=OP.add)
            # p0 = exp(kappa*cos - kappa + lnC)
            nc.scalar.activation(Q[:], h[:], AF.Exp, bias=BEXP[:],
                                 scale=KAPPA)

            # ---- residual Taylor on p0 (commutes with the bit applies):
            # V = Q + rdk_k*(X V), k=deg_r..1, as narrow matvec groups that
            # hide inside the prelude (one per round from the second round
            # on; leftovers drain under the S transpose).  The k==1 step
            # lands the evolved p0 straight in the MQ tile's Q slots.
            S = mpool.tile([P, NCH * W], F32R, tag="M")
            tay = {"V": (Q, BL, 0), "k": deg_r}

            def taylor_step():
                if tay["k"] < 1:
                    return
                k = tay["k"]
                Vt, vstr, voff = tay["V"]
                Vn = None if k == 1 else vpool.tile([P, QW], F32R, tag="V")
                for i in range(NCH):
                    ps = pss.tile([P, BL], F32, tag="ap", bufs=3)
                    for c in range(NCH):
                        nc.tensor.matmul(
                            ps[:], lhsT=xt_blk(c, i),
                            rhs=Vt[:, c * vstr + voff: c * vstr + voff + BL],
                            start=(c == 0), stop=(c == NCH - 1))
                    vs = (S[:, i * W + N: (i + 1) * W] if k == 1
                          else Vn[:, i * BL:(i + 1) * BL])
                    e_tt(vs, ps[:], RDK[:, (k - 1) * BL: k * BL], OP.mult,
                         "dve")
                    e_tt(vs, vs, Q[:, i * BL:(i + 1) * BL], OP.add, "pool")
                tay["V"] = (Vn, BL, 0)
                tay["k"] = k - 1

            # ---- prelude: Horner form of the Taylor S = sum X^j/j! --------
            # G_k = I + (X/k) G_{k+1}, k = deg_p-1..1; S = G_1.  The +k*I
            # rides the PE as one extra accumulate-matmul per chunk (host
            # sends k-scaled identities) and the 1/k folds into the psum
            # copy, so the prelude has NO elementwise adds at all.  The
            # final round's copy lands G_1 straight in the MQ tile's S
            # slots; M tiles are MQ-shaped ([M_c | Q_c] per chunk) so the
            # bit-applies merge into the squaring matmuls.
            # the first round folds G_{deg_p} in: psum = X@X + deg_p*X +
            # (deg_p-1)*deg_p*I, scaled by 1/((deg_p-1)*deg_p), which equals
            # G_{deg_p-1} -- so no G init chain gates the first matmuls
            G = None
            for ridx, k in enumerate(range(deg_p - 1, 0, -1)):
                first = G is None
                lastr = k == 1
                Gn = (S if lastr
                      else mpool.tile([P, NCH * N], F32R, tag="T"))
                for i in range(NCH):
                    ps = psb.tile([P, N], F32, tag="sq", bufs=3)
                    for c in range(NCH):
                        nc.tensor.matmul(
                            ps[:], lhsT=xt_blk(c, i),
                            rhs=(xn(c) if first
                                 else G[:, c * N: (c + 1) * N]),
                            start=(c == 0), stop=False)
                    # +v*I rides as a [P,P] sub-range accumulate: the
                    # identity slab is zero outside column-block i
                    if first:
                        nc.tensor.matmul(
                            ps[:], lhsT=ke_blk(deg_p), rhs=xn(i),
                            start=False, stop=False)
                        nc.tensor.matmul(
                            ps[:, i * P: (i + 1) * P],
                            lhsT=ke_blk((deg_p - 1) * deg_p), rhs=E120,
                            start=False, stop=True, skip_group_check=True)
                        scale = 1.0 / ((deg_p - 1) * deg_p)
                    else:
                        nc.tensor.matmul(
                            ps[:, i * P: (i + 1) * P], lhsT=ke_blk(k),
                            rhs=E120,
                            start=False, stop=True, skip_group_check=True)
                        scale = None if k == 1 else 1.0 / k
                    dst = (Gn[:, i * W: i * W + N] if lastr
                           else Gn[:, i * N: (i + 1) * N])
                    e_copy(dst, ps[:], "act", scale=scale)
                G = Gn
                if ridx >= 1:
                    # p0 is ready by the end of round two; the PE is in-order
                    # so a step emitted earlier would stall the round matmuls
                    taylor_step()

            ST = mpool.tile([P, NCH * N], F32R, tag="MT")

            def transpose_mq(MTt, Mt):
                # phase ib: transpose the 3 column-blocks of S chunk ib into
                # one [P, N] psum, then a single strided copy into MT's
                # column-block ib of every chunk.  Phase 0 lands first so the
                # next level's first matmul group unblocks early.
                for ib in range(NCH):
                    pst = psb.tile([P, N], F32R, tag="tr", bufs=2)
                    for cp in range(NCH):
                        nc.tensor.transpose(
                            pst[:, cp * P:(cp + 1) * P],
                            Mt[:, ib * W + cp * P: ib * W + cp * P + P],
                            E120[:],
                        )
                    dst = MTt[:].rearrange(
                        "p (c n) -> p c n", c=NCH)[:, :, ib * P:(ib + 1) * P]
                    pv3 = pst[:].rearrange("p (c n) -> p c n", c=NCH)
                    e_copy(dst, pv3, ("act", "act", "dve")[ib])

            transpose_mq(ST, S)
            M, MT = S, ST

            def mt_blk(c, i):
                return MT[:, c * N + i * P: c * N + (i + 1) * P]

            def blend_q(dst, old, new_ps, bit):
                # dst = old + msk*(new - old); copy_predicated can't write
                # fp32r-typed tiles, so blend arithmetically (msk is 0/1)
                t = tpool.tile([P, BL], F32, tag="t3")
                e_tt(t[:], new_ps, old, OP.subtract, "dve")
                e_tt(t[:], t[:], MSK[:, bit * BL:(bit + 1) * BL], OP.mult,
                     "pool")
                e_tt(dst, old, t[:], OP.add, "pool")

            def square(Mc, MTc, MTc_blk, bit=None):
                # Sn = Mc@Mc; if bit is not None also compute Mc@Q (merged
                # columns) and blend it into Sn's Q slot under the bit mask.
                Sn = mpool.tile([P, NCH * W], F32R, tag="M")
                STn = mpool.tile([P, NCH * N], F32R, tag="MT")
                wid = N if bit is None else W
                for i in range(NCH):
                    ps = psb.tile([P, wid], F32, tag="sq", bufs=3)
                    mm_group(ps, MTc_blk, Mc, i, wid, rhs_stride=W)
                    e_copy(Sn[:, i * W: i * W + N], ps[:, :N],
                           ("act", "dve", "dve")[i])
                    if bit is not None:
                        blend_q(Sn[:, i * W + N: (i + 1) * W],
                                Mc[:, i * W + N: (i + 1) * W],
                                ps[:, N:W], bit)
                transpose_mq(STn, Sn)
                return Sn, STn

            # drain remaining taylor steps; the k==1 step lands the
            # evolved p0 directly in the MQ tile's Q slots
            while tay["k"] >= 1:
                taylor_step()

            # ---- merged bit applies + chain squarings ---------------------
            # level j squares M (= expm(2^j T0 A)) and applies bit j of the
            # quantized delay to Q in the same matmul set.  The top TWO bits
            # need no further squaring: bit k-2 is a single apply of M_{k-2}
            # and bit k-1 a double apply (M_{k-1} Q = M_{k-2} (M_{k-2} Q)),
            # which is ~2x cheaper than materializing M_{k-1}.
            for j in range(k_bits - 2):
                M, MT = square(M, MT, mt_blk, bit=j)

            # Z: per chunk [Q_c | z1_c | z2_c | z3_c], z_r = M^r Q with
            # M = M_{k-2}: a blend-free branch tree over the top two bits
            # (masked blends between narrow applies were pure semaphore
            # latency); the host picks the branch per sample.
            ZW = 4 * BL
            Z = qpool.tile([P, NCH * ZW], F32R, tag="z")
            z3ps = []
            for i in range(NCH):
                e_copy(Z[:, i * ZW: i * ZW + BL],
                       M[:, i * W + N: i * W + N + BL],
                       ("dve", "act", "dve")[i])
            for r in range(1, 4):
                for i in range(NCH):
                    ps = pss.tile([P, BL], F32, tag="ap", bufs=3)
                    for c in range(NCH):
                        nc.tensor.matmul(
                            ps[:],
                            lhsT=mt_blk(c, i),
                            rhs=(M[:, c * W + N: c * W + N + BL] if r == 1
                                 else Z[:, c * ZW + (r - 1) * BL:
                                        c * ZW + r * BL]),
                            start=(c == 0), stop=(c == NCH - 1),
                        )
                    if r < 3:
                        e_copy(Z[:, i * ZW + r * BL: i * ZW + (r + 1) * BL],
                               ps[:], ("dve", "act", "dve")[i])
                    else:
                        z3ps.append(ps)   # z3 never needs SBUF

            # ---- stacked selection of all four branches; the branch pick
            # AND the log/mean loss tail run on host (it has the bits).
            # The Q|z1|z2 columns accumulate while z3 is still in flight;
            # z3's part multiplies straight out of PSUM.
            sel = pss.tile([1, ZW], F32, tag="ap", bufs=3)
            W3 = 3 * BL
            for c in range(NCH):
                tmp = tpool.tile([P, ZW], F32, tag="t2")
                e_tt(tmp[:, 0:W3], Z[:, c * ZW: c * ZW + W3],
                     OH4[:, c * ZW: c * ZW + W3], OP.mult,
                     ("dve", "pool", "dve")[c])
                nc.tensor.matmul(sel[:, 0:W3], lhsT=ONES[:],
                                 rhs=tmp[:, 0:W3], start=(c == 0),
                                 stop=(c == NCH - 1), skip_group_check=True)
            for c in range(NCH):
                t3 = tpool.tile([P, BL], F32, tag="t3")
                e_tt(t3[:], z3ps[c][:],
                     OH4[:, c * ZW + W3: (c + 1) * ZW], OP.mult, "dve")
                nc.tensor.matmul(sel[:, W3:ZW], lhsT=ONES[:], rhs=t3[:],
                                 start=(c == 0), stop=(c == NCH - 1),
                                 skip_group_check=True)
            OUT = tpool.tile([1, ZW], F32, tag="r0")
            nc.vector.tensor_copy(OUT[:], sel[:])
            nc.sync.dma_start(d_out[:], OUT[:])

    nc.compile()
    return nc


def _host_prep(c_mesh, gtheta, sigma_diff, init_color, delay_t, report_color):
    """Host-side glue: operator assembly (replicating reference f32 ops),
    plan selection, and per-core index/bit/layout arrays."""
    f32 = np.float32
    c = np.asarray(c_mesh, dtype=f32)
    g = np.asarray(gtheta, dtype=f32)
    s = np.asarray(sigma_diff, dtype=f32)[0]
    init = np.asarray(init_color, dtype=f32)
    t = np.asarray(delay_t, dtype=f32)
    rep = np.asarray(report_color, dtype=f32)

    d = (c[1] - c[0]).astype(f32)
    eye = np.eye(N, dtype=f32)
    up = np.roll(eye, -1, axis=1)
    dn = np.roll(eye, 1, axis=1)
    D1 = ((up - dn) / (f32(2.0) * d)).astype(f32)
    D2 = ((up - f32(2.0) * eye + dn) / (d * d)).astype(f32)
    A = ((s ** f32(2.0)) / f32(2.0) * D2 - D1 * g[None, :]).astype(f32)

    anorm = np.abs(A.astype(np.float64)).sum(axis=1).max()
    k_bits, deg_p, deg_r = plan = _plan(anorm)
    T0 = T_MAX / (1 << k_bits)
    X = (A * f32(T0)).astype(f32)
    XT = np.ascontiguousarray(X.T)

    m = np.floor(t.astype(np.float64) / T0).astype(np.int64)
    m = np.clip(m, 0, (1 << k_bits) - 1)
    r = (t.astype(np.float64) - m * T0) / T0  # in X = T0*A units
    bits = ((m[:, None] >> np.arange(k_bits)[None, :]) & 1)     # [B, K]
    idx = np.argmin(np.abs(c[None, :] - rep[:, None]), axis=1)

    # X|XT image, per-chunk interleaved: [XNc | XTc] so the first prelude
    # matmul only needs the first chunk DMA
    xe = np.empty((P, NCH * 2 * N), f32)
    for ci in range(NCH):
        xe[:, ci * 2 * N: ci * 2 * N + N] = X[ci * P:(ci + 1) * P, :]
        xe[:, ci * 2 * N + N: (ci + 1) * 2 * N] = XT[ci * P:(ci + 1) * P, :]
    ke_vals = list(range(1, deg_p - 1)) + [deg_p, (deg_p - 1) * deg_p]
    if 1 not in ke_vals:
        ke_vals = [1] + ke_vals
    ke = np.empty((P, len(ke_vals) * P), f32)
    for j, v in enumerate(ke_vals):
        ke[:, j * P: (j + 1) * P] = v * np.eye(P, dtype=f32)

    # CM3[p, c*BL+b] = c_mesh[c*P+p]
    cm3 = np.broadcast_to(
        c.reshape(NCH, P).T[:, :, None], (P, NCH, BL)).reshape(P, NCH * BL)

    shared = {
        "xe": xe,
        "ke": ke,
    }
    in_maps = []
    for core in range(NCORES):
        sl = slice(core * BL, (core + 1) * BL)
        irep = np.broadcast_to(np.tile(init[sl], NCH)[None, :],
                               (P, NCH * BL)).astype(f32)
        pv = np.concatenate([cm3, irep], axis=1).astype(f32)
        nmsk = max(k_bits - 2, 0)
        msk = np.broadcast_to(
            bits[sl, :nmsk].T.reshape(1, nmsk * BL), (P, nmsk * BL)
        ).astype(f32)  # bit j at [j*BL:(j+1)*BL]
        rdk = np.empty((deg_r, BL), f32)
        for k in range(1, deg_r + 1):
            rdk[k - 1] = (r[sl] / k).astype(f32)
        rdk = np.broadcast_to(
            rdk.reshape(1, deg_r * BL), (P, deg_r * BL)).astype(f32)
        oh = np.zeros((NCH, P, BL), f32)
        for b, ix in enumerate(idx[sl]):
            oh[ix // P, ix % P, b] = 1.0
        # per chunk, replicated 4x to select all branch-tree rows at once
        oh4 = np.ascontiguousarray(
            np.tile(oh, (1, 1, 4)).transpose(1, 0, 2).reshape(P, NCH * 4 * BL))
        aux = np.concatenate([rdk, msk, oh4], axis=1).astype(f32)
        in_maps.append(dict(shared, pv=pv, aux=aux))
    return plan, in_maps


def _get_nc(plan):
    if plan not in _COMPILED:
        _COMPILED[plan] = _build_bass(*plan)
    return _COMPILED[plan]


def kernel(**inputs):
    from concourse.bass_utils import run_bass_kernel_spmd

    plan, in_maps = _host_prep(
        inputs["c_mesh"], inputs["gtheta"], inputs["sigma_diff"],
        inputs["init_color"], inputs["delay_t"], inputs["report_color"],
    )
    nc = _get_nc(plan)
    res = run_bass_kernel_spmd(nc, in_maps, list(range(NCORES)))
    # branch r of the tree (M^r Q, r = top two time-bits) at [r*BL + b]
    sAB = np.stack(
        [np.asarray(res.results[k]["sel"]).reshape(4, BL)
         for k in range(NCORES)]
    )  # [NCORES, 4, BL]
    k_bits = plan[0]
    t = np.asarray(inputs["delay_t"], dtype=np.float32)
    T0 = T_MAX / (1 << k_bits)
    m = np.clip(np.floor(t.astype(np.float64) / T0).astype(np.int64),
                0, (1 << k_bits) - 1)
    v = ((m >> (k_bits - 2)) & 3).reshape(NCORES, BL)
    sel = np.take_along_axis(sAB, v[:, None, :], axis=1)[:, 0, :].reshape(-1)
    terms = np.log(np.maximum(sel.astype(np.float64), 0.0) + EPS)
    loss = -np.mean(terms)
    return np.asarray(loss, dtype=np.float32)


# revision 93
# speedup vs baseline: 1.0019x; 1.0019x over previous
"""Trainium2 Bass kernel for the circular drift-diffusion loss (batched expm).

Reference computes  loss = -mean_b log(relu(e_{idx_b}^T expm(t_b*A) p0_b) + eps)
with A a fixed 360x360 circular advection-diffusion operator, t_b in [0,1000),
p0_b a von Mises density, over a batch of 256.

Algorithm (per core; batch sharded 32/core over 8 cores):
  * Quantize t_b = m_b*T0 + r_b with T0 = 1000/2^K, m_b < 2^K.
  * Build the propagator chain M_j = expm(2^j*T0*A) by repeated squaring.
    The prelude evaluates the Taylor of expm(T0*A) in Horner form
    G_k = I + (X/k)G_{k+1}: the +k*I terms ride the PE as accumulate-
    matmuls against host-sent scaled identities and the 1/k scales fold
    into the psum->SBUF copies, so the prelude needs no elementwise adds.
  * ALL wide matmuls run in fp32r: 1 PE row/cycle (vs 4 for fp32) when
    the moving dim >= 256.  Its rounding noise through the chain is
    ~1e-3 relative on the density, i.e. ~1e-4 on the log-loss -- two
    orders inside the 2e-2 budget -- and the Taylor tolerances are
    relaxed to match (which also shortens the chain to K=7).
  * Bits 0..K-3 of m_b apply as masked batched matvecs merged into the
    squaring matmuls (32 extra moving columns); the blends are
    arithmetic (old + msk*(new-old)) since copy_predicated can't write
    fp32r.  The top TWO bits are blend-free: the kernel emits the branch
    tree Q, MQ, M^2Q, M^3Q (M = M_{K-2}), selects all four, and the host
    picks per sample -- blends between narrow applies were pure latency.
  * Residual: Q <- Taylor_DEG_R(r_b A) Q with per-sample r folded into
    host-precomputed coefficient tables; the narrow matvec steps hide
    inside the prelude rounds.
  * p0 built on device as one [P, NCH*BL]-wide op chain on a single
    engine (min-of-squares fold, Estrin cos poly, Exp activation);
    selection via one-hot + PE column-sum.  The branch pick and the
    log/mean loss tail run on host (O(B) glue).
Scheduling notes: dummy warm-up matmuls burn the PE's 2x-slow p-state
ramp inside the initial DMA shadow; loads are split across both HWDGE
queues and the gpsimd SWDGE ring because DMA transfers serialize per
queue; elementwise ops are hand-pinned to DVE/ACT/Pool so the copies
that gate each level land on an engine that is free at that moment.
"""

import math

import numpy as np

# ---------------- static problem constants (hardcoded per contract) ----------
N = 360            # color mesh size
P = 120            # partition chunk (N = 3*P)
NCH = 3            # chunks
B = 256            # total batch
NCORES = 8
BL = B // NCORES   # per-core batch
QW = NCH * BL      # width of a full Q block
T_MAX = 1000.0
KAPPA = 400.0      # 1/SIGMA_INIT^2
EPS = 1e-5
TWO_PI = 6.283185307179586
# ln(1/(2*pi*i0e(400)))  [i0e(400) = 0.019953356281939987]
LNC = 2.076480848703078
# cos(sqrt(u)) on u in [0, pi^2] (|delta| folded to [0,pi]), power basis c0..c8
COS_COEF = [1.00000000e+00, -5.00000000e-01, 4.16666666e-02, -1.38888885e-03,
            2.48015646e-05, -2.75566515e-07, 2.08651966e-09, -1.13535474e-11,
            4.13131734e-14]

_COMPILED = {}


def _taylor_deg(x, tol, lo):
    """Smallest d with x^(d+1)/(d+1)! < tol."""
    d = lo
    term = x ** (d + 1) / math.factorial(d + 1)
    while term > tol and d < 40:
        d += 1
        term *= x / (d + 1)
    return d


def _plan(anorm):
    """Choose (k_bits, deg_p, deg_r) from ||A||_inf.  The time grid is
    T0 = T_MAX/2^k_bits; every squaring level applies one bit of the
    quantized delay.  Tolerances sit just under the fp32r rounding noise
    (~1e-3 through the chain), which the 2e-2 rel-err budget dwarfs."""
    xa = T_MAX * float(anorm)
    if xa <= 0.0:
        return 2, 4, 3
    k0 = max(2, min(16, math.ceil(math.log2(max(xa / 0.9, 2.0)))))

    def degrees(k):
        x0 = xa / (1 << k)
        # tolerances are RELATIVE error on the propagated density; the log
        # in the loss divides that by |loss|~10, so a few percent is still
        # an order of magnitude inside the 2e-2 budget.  Prelude truncation
        # is amplified ~2^(k/2) through the squarings; the residual Taylor
        # is applied once (no amplification).
        tol_p = max(min(2.5e-2 / 2 ** (k / 2), 2e-3), 5e-8)
        return _taylor_deg(x0, tol_p, 2), _taylor_deg(x0, 1e-2, 2)

    # pick k by explicit cost minimization with measured per-stage costs
    # (chain level ~3.0us, prelude step ~1.8us, taylor step ~0.3us wall)
    best = None
    for k in range(max(2, k0 - 2), min(16, k0 + 2) + 1):
        dp, dr = degrees(k)
        cost = (k - 1) * 3.0 + (dp - 1) * 1.8 + dr * 0.3
        if best is None or cost < best[0]:
            best = (cost, k, dp, dr)
    _, k, deg_p, deg_r = best
    return k, deg_p, deg_r


def _build_bass(k_bits, deg_p, deg_r):
    """Construct the Bass program (SPMD; identical on all 8 cores)."""
    import concourse.tile as tile
    from concourse import bacc, mybir

    F32 = mybir.dt.float32
    F32R = mybir.dt.float32r
    AF = mybir.ActivationFunctionType
    OP = mybir.AluOpType

    nc = bacc.Bacc("TRN2", target_bir_lowering=False, debug=False)

    NMSK = max(k_bits - 2, 0)
    AUXW = (deg_r + NMSK + 4 * NCH) * BL

    d_xe = nc.dram_tensor("xe", [P, NCH * N], F32R,
                          kind="ExternalInput").ap()   # X row-chunks
    ke_vals = list(range(1, deg_p - 1)) + [deg_p, (deg_p - 1) * deg_p]
    if 1 not in ke_vals:
        ke_vals = [1] + ke_vals
    ke_idx = {v: i for i, v in enumerate(ke_vals)}
    d_ke = nc.dram_tensor("ke", [P, len(ke_vals) * P], F32R,
                          kind="ExternalInput").ap()   # v*I_P per slot
    d_pv = nc.dram_tensor("pv", [P, 2 * QW], F32,
                          kind="ExternalInput").ap()   # [CM3|IREP]
    d_aux = nc.dram_tensor("aux", [P, AUXW], F32,
                           kind="ExternalInput").ap()  # [RDK|MSK|OH]
    d_out = nc.dram_tensor("sel", [1, 4 * BL], F32,
                           kind="ExternalOutput").ap()

    with tile.TileContext(nc) as tc:
        with (
            tc.tile_pool(name="const", bufs=1) as cpool,
            tc.tile_pool(name="mats", bufs=4) as mpool,
            tc.tile_pool(name="qp", bufs=2) as qpool,
            tc.tile_pool(name="vp", bufs=3) as vpool,
            tc.tile_pool(name="tp", bufs=4) as tpool,
            tc.tile_pool(name="psb", bufs=5, space="PSUM") as psb,
            tc.tile_pool(name="pss", bufs=3, space="PSUM") as pss,
        ):
            # ---- engine helpers: explicit pinning (GPSIMD can't read PSUM,
            # ACT can't do tensor_tensor; criticial-path copies go to the
            # engine that is free at that point of each level)
            def e_copy(dst, src, eng, scale=None):
                if scale is not None:
                    if eng == "dve":
                        nc.vector.tensor_scalar(dst, src, scale, None,
                                                op0=OP.mult)
                    elif eng == "act":
                        nc.scalar.mul(dst, src, scale)
                    else:
                        nc.gpsimd.tensor_scalar(dst, src, scale, None,
                                                op0=OP.mult)
                else:
                    if eng == "dve":
                        nc.vector.tensor_copy(dst, src)
                    elif eng == "act":
                        nc.scalar.copy(dst, src)
                    else:
                        nc.gpsimd.tensor_copy(dst, src)

            def e_tt(dst, a, b, op, eng):
                (nc.vector if eng == "dve" else nc.gpsimd).tensor_tensor(
                    dst, a, b, op=op)

            # ---- constants.  HWDGE costs ~625ns of serialized ring per
            # DMA, so the critical loads (pv for p0, then the X|XT chunks)
            # go there in need-order while everything else rides the
            # separate software-DGE ring (gpsimd-issued).
            # transfers serialize per hardware queue, so spread the loads
            # over all three HWDGE queues (SP/DVE/ACT): the X|XT chunks land
            # concurrently ~4.4us in instead of staggering 1us apart
            # X^T is built ON DEVICE (PE transposes in the warmup shadow):
            # only X rides the serialized DMA transfer FIFO.  The identity
            # (ke slot 0) goes first -- the transposes need it.
            PV = cpool.tile([P, 2 * QW], F32, tag="pv")
            CXT = cpool.tile([P, NCH * N], F32R, tag="cxt")
            KE = cpool.tile([P, len(ke_vals) * P], F32R, tag="ke")
            nc.sync.dma_start(KE[:, 0:P], d_ke[:, 0:P])
            nc.sync.dma_start(CXT[:, 0:N], d_xe[:, 0:N])
            nc.scalar.dma_start(CXT[:, N: 2 * N], d_xe[:, N: 2 * N])
            nc.gpsimd.dma_start(PV[:], d_pv[:])
            nc.sync.dma_start(CXT[:, 2 * N: 3 * N], d_xe[:, 2 * N: 3 * N])
            if len(ke_vals) > 1:
                nc.gpsimd.dma_start(KE[:, P:], d_ke[:, P:])
            AUX = cpool.tile([P, AUXW], F32, tag="aux")
            nc.gpsimd.dma_start(AUX[:], d_aux[:])

            def ke_blk(v):
                return KE[:, ke_idx[v] * P: (ke_idx[v] + 1) * P]
            # fp32r identity for transposes (bf16 would rate 1.0 cyc/row vs
            # 1.5 but lowers to Ldweights+matmul pairs that stall the PE)
            E120 = KE[:, 0:P]   # ke slot 0 is 1*I

            # warm-up matmuls: the PE runs its first ~3us at the mid p-state
            # (2x cycle time); burning that ramp on dummies while the DMAs
            # land makes the real prelude run at full clock
            W0 = cpool.tile([P, N], F32, tag="w0m")
            nc.vector.memset(W0[:], 0.0)
            for wmw in (N, N):
                wps = psb.tile([P, N], F32, tag="sq", bufs=3)
                nc.tensor.matmul(wps[:, 0:wmw], lhsT=W0[:, 0:P],
                                 rhs=W0[:, 0:wmw], start=True, stop=True)

            XTD = cpool.tile([P, NCH * N], F32R, tag="xtd")

            def xn(c):
                return CXT[:, c * N: (c + 1) * N]

            def xt_blk(c, i):
                return XTD[:, c * N + i * P: c * N + (i + 1) * P]

            RDK = AUX[:, 0: deg_r * BL]
            MSK = AUX[:, deg_r * BL: (deg_r + NMSK) * BL]
            OH4 = AUX[:, (deg_r + NMSK) * BL: AUXW]

            ONES = cpool.tile([P, 1], F32, tag="ones")
            nc.vector.memset(ONES[:], 1.0)
            BEXP = cpool.tile([P, 1], F32, tag="bexp")
            nc.vector.memset(BEXP[:], LNC - KAPPA)

            W = N + BL  # merged chunk width: [M_c | Q_c]

            def mm_group(ps, lhsT_of, rhs_tile, i, rhs_w, rhs_stride=None):
                rs = rhs_w if rhs_stride is None else rhs_stride
                for c in range(NCH):
                    nc.tensor.matmul(
                        ps[:],
                        lhsT=lhsT_of(c, i),
                        rhs=rhs_tile[:, c * rs: c * rs + rhs_w],
                        start=(c == 0), stop=(c == NCH - 1),
                    )

            # ---- p0 (von Mises) as one [P, QW]-wide op chain.  Latency
            # matters (p0 gates the residual Taylor steps hidden inside the
            # prelude), so: u = min(d^2, (d-2pi)^2, (d+2pi)^2) needs no Abs
            # round-trip, and cos(sqrt(u)) evaluates Estrin-style (depth ~7)
            # split across DVE and Pool.
            CM3 = PV[:, 0:QW]
            IREP = PV[:, QW:2 * QW]
            Q = qpool.tile([P, QW], F32R, tag="q")
            # one engine (DVE) end to end: no cross-engine semaphore hops on
            # the latency spine, and DVE's prelude work (fused S-updates)
            # can lag since S is only read at the chain's first level
            V = nc.vector
            dl = tpool.tile([P, QW], F32, tag="w0")
            V.tensor_tensor(dl[:], IREP, CM3, op=OP.subtract)
            bm = tpool.tile([P, QW], F32, tag="w1")
            V.tensor_scalar(bm[:], dl[:], 1.0, -TWO_PI,
                            op0=OP.mult, op1=OP.add)
            cp_ = tpool.tile([P, QW], F32, tag="w2")
            V.tensor_scalar(cp_[:], dl[:], 1.0, TWO_PI,
                            op0=OP.mult, op1=OP.add)
            a2 = tpool.tile([P, QW], F32, tag="w3")
            V.tensor_tensor(a2[:], dl[:], dl[:], op=OP.mult)
            b2 = tpool.tile([P, QW], F32, tag="w1")
            V.tensor_tensor(b2[:], bm[:], bm[:], op=OP.mult)
            c2 = tpool.tile([P, QW], F32, tag="w2")
            V.tensor_tensor(c2[:], cp_[:], cp_[:], op=OP.mult)
            u = tpool.tile([P, QW], F32, tag="w0")
            V.tensor_tensor(u[:], b2[:], c2[:], op=OP.min)
            V.tensor_tensor(u[:], u[:], a2[:], op=OP.min)
            # Estrin: pairs via fused (x*s0 + s1), then combine with powers
            u2 = tpool.tile([P, QW], F32, tag="w1")
            V.tensor_tensor(u2[:], u[:], u[:], op=OP.mult)
            p01 = tpool.tile([P, QW], F32, tag="w2")
            V.tensor_scalar(p01[:], u[:], COS_COEF[1], COS_COEF[0],
                            op0=OP.mult, op1=OP.add)
            p23 = tpool.tile([P, QW], F32, tag="w3")
            V.tensor_scalar(p23[:], u[:], COS_COEF[3], COS_COEF[2],
                            op0=OP.mult, op1=OP.add)
            p45 = tpool.tile([P, QW], F32, tag="w4")
            V.tensor_scalar(p45[:], u[:], COS_COEF[5], COS_COEF[4],
                            op0=OP.mult, op1=OP.add)
            p67 = tpool.tile([P, QW], F32, tag="w5")
            V.tensor_scalar(p67[:], u[:], COS_COEF[7], COS_COEF[6],
                            op0=OP.mult, op1=OP.add)
            u4 = tpool.tile([P, QW], F32, tag="w6")
            V.tensor_tensor(u4[:], u2[:], u2[:], op=OP.mult)
            # q67 = p67 + c8*u2 ; hi = p45 + q67*u2 ; lo = p01 + p23*u2
            q67 = tpool.tile([P, QW], F32, tag="w1")
            V.tensor_scalar(q67[:], u2[:], COS_COEF[8], None, op0=OP.mult)
            V.tensor_tensor(q67[:], q67[:], p67[:], op=OP.add)
            V.tensor_tensor(q67[:], q67[:], u2[:], op=OP.mult)
            hi = tpool.tile([P, QW], F32, tag="w4")
            V.tensor_tensor(hi[:], p45[:], q67[:], op=OP.add)
            lo = tpool.tile([P, QW], F32, tag="w2")
            V.tensor_tensor(p23[:], p23[:], u2[:], op=OP.mult)
            V.tensor_tensor(lo[:], p01[:], p23[:], op=OP.add)
            h = tpool.tile([P, QW], F32, tag="w0")
            V.tensor_tensor(h[:], hi[:], u4[:], op=OP.mult)
            V.tensor_tensor(h[:], h[:], lo[:], op=OP.add)
            # p0 = exp(kappa*cos - kappa + lnC)
            nc.scalar.activation(Q[:], h[:], AF.Exp, bias=BEXP[:],
                                 scale=KAPPA)

            # ---- residual Taylor on p0 (commutes with the bit applies):
            # V = Q + rdk_k*(X V), k=deg_r..1, as narrow matvec groups that
            # hide inside the prelude (one per round from the second round
            # on; leftovers drain under the S transpose).  The k==1 step
            # lands the evolved p0 straight in the MQ tile's Q slots.
            S = mpool.tile([P, NCH * W], F32R, tag="M")
            tay = {"V": (Q, BL, 0), "k": deg_r}

            def taylor_step():
                if tay["k"] < 1:
                    return
                k = tay["k"]
                Vt, vstr, voff = tay["V"]
                Vn = None if k == 1 else vpool.tile([P, QW], F32R, tag="V")
                for i in range(NCH):
                    ps = pss.tile([P, BL], F32, tag="ap", bufs=3)
                    for c in range(NCH):
                        nc.tensor.matmul(
                            ps[:], lhsT=xt_blk(c, i),
                            rhs=Vt[:, c * vstr + voff: c * vstr + voff + BL],
                            start=(c == 0), stop=(c == NCH - 1))
                    vs = (S[:, i * W + N: (i + 1) * W] if k == 1
                          else Vn[:, i * BL:(i + 1) * BL])
                    e_tt(vs, ps[:], RDK[:, (k - 1) * BL: k * BL], OP.mult,
                         "dve")
                    e_tt(vs, vs, Q[:, i * BL:(i + 1) * BL], OP.add, "pool")
                tay["V"] = (Vn, BL, 0)
                tay["k"] = k - 1

            # ---- prelude: Horner form of the Taylor S = sum X^j/j! --------
            # G_k = I + (X/k) G_{k+1}, k = deg_p-1..1; S = G_1.  The +k*I
            # rides the PE as one extra accumulate-matmul per chunk (host
            # sends k-scaled identities) and the 1/k folds into the psum
            # copy, so the prelude has NO elementwise adds at all.  The
            # final round's copy lands G_1 straight in the MQ tile's S
            # slots; M tiles are MQ-shaped ([M_c | Q_c] per chunk) so the
            # bit-applies merge into the squaring matmuls.
            # the first round folds G_{deg_p} in: psum = X@X + deg_p*X +
            # (deg_p-1)*deg_p*I, scaled by 1/((deg_p-1)*deg_p), which equals
            # G_{deg_p-1} -- so no G init chain gates the first matmuls
            def build_xt(i):
                # XTD block (c,i) = transpose of X chunk i's column-block c
                pst = psb.tile([P, N], F32R, tag="tr", bufs=2)
                for c in range(NCH):
                    nc.tensor.transpose(
                        pst[:, c * P:(c + 1) * P],
                        CXT[:, i * N + c * P: i * N + (c + 1) * P],
                        E120[:])
                dst = XTD[:].rearrange(
                    "p (c n) -> p c n", c=NCH)[:, :, i * P:(i + 1) * P]
                e_copy(dst, pst[:].rearrange("p (c n) -> p c n", c=NCH),
                       ("dve", "act", "dve")[i])

            build_xt(0)
            build_xt(1)

            G = None
            for ridx, k in enumerate(range(deg_p - 1, 0, -1)):
                first = G is None
                lastr = k == 1
                Gn = (S if lastr
                      else mpool.tile([P, NCH * N], F32R, tag="T"))
                for i in range(NCH):
                    if first and i == 1:
                        # phase 2's transposes slot in behind group 0 so
                        # the PE isn't head-of-line blocked on chunk 2
                        build_xt(2)
                    ps = psb.tile([P, N], F32, tag="sq", bufs=3)
                    for c in range(NCH):
                        nc.tensor.matmul(
                            ps[:], lhsT=xt_blk(c, i),
                            rhs=(xn(c) if first
                                 else G[:, c * N: (c + 1) * N]),
                            start=(c == 0), stop=False)
                    # +v*I rides as a [P,P] sub-range accumulate: the
                    # identity slab is zero outside column-block i
                    if first:
                        nc.tensor.matmul(
                            ps[:], lhsT=ke_blk(deg_p), rhs=xn(i),
                            start=False, stop=False)
                        nc.tensor.matmul(
                            ps[:, i * P: (i + 1) * P],
                            lhsT=ke_blk((deg_p - 1) * deg_p), rhs=E120,
                            start=False, stop=True, skip_group_check=True)
                        scale = 1.0 / ((deg_p - 1) * deg_p)
                    else:
                        nc.tensor.matmul(
                            ps[:, i * P: (i + 1) * P], lhsT=ke_blk(k),
                            rhs=E120,
                            start=False, stop=True, skip_group_check=True)
                        scale = None if k == 1 else 1.0 / k
                    dst = (Gn[:, i * W: i * W + N] if lastr
                           else Gn[:, i * N: (i + 1) * N])
                    e_copy(dst, ps[:], "act", scale=scale)
                G = Gn
                if ridx >= 1:
                    # p0 is ready by the end of round two; the PE is in-order
                    # so a step emitted earlier would stall the round matmuls
                    taylor_step()

            ST = mpool.tile([P, NCH * N], F32R, tag="MT")

            def transpose_mq(MTt, Mt):
                # phase ib: transpose the 3 column-blocks of S chunk ib into
                # one [P, N] psum, then a single strided copy into MT's
                # column-block ib of every chunk.  Phase 0 lands first so the
                # next level's first matmul group unblocks early.
                for ib in range(NCH):
                    pst = psb.tile([P, N], F32R, tag="tr", bufs=2)
                    for cp in range(NCH):
                        nc.tensor.transpose(
                            pst[:, cp * P:(cp + 1) * P],
                            Mt[:, ib * W + cp * P: ib * W + cp * P + P],
                            E120[:],
                        )
                    dst = MTt[:].rearrange(
                        "p (c n) -> p c n", c=NCH)[:, :, ib * P:(ib + 1) * P]
                    pv3 = pst[:].rearrange("p (c n) -> p c n", c=NCH)
                    e_copy(dst, pv3, ("act", "act", "dve")[ib])

            transpose_mq(ST, S)
            M, MT = S, ST

            def mt_blk(c, i):
                return MT[:, c * N + i * P: c * N + (i + 1) * P]

            def blend_q(dst, old, new_ps, bit):
                # dst = old + msk*(new - old); copy_predicated can't write
                # fp32r-typed tiles, so blend arithmetically (msk is 0/1)
                t = tpool.tile([P, BL], F32, tag="t3")
                e_tt(t[:], new_ps, old, OP.subtract, "dve")
                e_tt(t[:], t[:], MSK[:, bit * BL:(bit + 1) * BL], OP.mult,
                     "pool")
                e_tt(dst, old, t[:], OP.add, "pool")

            def square(Mc, MTc, MTc_blk, bit=None):
                # Sn = Mc@Mc; if bit is not None also compute Mc@Q (merged
                # columns) and blend it into Sn's Q slot under the bit mask.
                Sn = mpool.tile([P, NCH * W], F32R, tag="M")
                STn = mpool.tile([P, NCH * N], F32R, tag="MT")
                wid = N if bit is None else W
                for i in range(NCH):
                    ps = psb.tile([P, wid], F32, tag="sq", bufs=3)
                    mm_group(ps, MTc_blk, Mc, i, wid, rhs_stride=W)
                    e_copy(Sn[:, i * W: i * W + N], ps[:, :N],
                           ("act", "dve", "dve")[i])
                    if bit is not None:
                        blend_q(Sn[:, i * W + N: (i + 1) * W],
                                Mc[:, i * W + N: (i + 1) * W],
                                ps[:, N:W], bit)
                transpose_mq(STn, Sn)
                return Sn, STn

            # drain remaining taylor steps; the k==1 step lands the
            # evolved p0 directly in the MQ tile's Q slots
            while tay["k"] >= 1:
                taylor_step()

            # ---- merged bit applies + chain squarings ---------------------
            # level j squares M (= expm(2^j T0 A)) and applies bit j of the
            # quantized delay to Q in the same matmul set.  The top TWO bits
            # need no further squaring: bit k-2 is a single apply of M_{k-2}
            # and bit k-1 a double apply (M_{k-1} Q = M_{k-2} (M_{k-2} Q)),
            # which is ~2x cheaper than materializing M_{k-1}.
            for j in range(k_bits - 2):
                M, MT = square(M, MT, mt_blk, bit=j)

            # Z: per chunk [Q_c | z1_c | z2_c | z3_c], z_r = M^r Q with
            # M = M_{k-2}: a blend-free branch tree over the top two bits
            # (masked blends between narrow applies were pure semaphore
            # latency); the host picks the branch per sample.
            ZW = 4 * BL
            Z = qpool.tile([P, NCH * ZW], F32R, tag="z")
            z3ps = []
            for i in range(NCH):
                e_copy(Z[:, i * ZW: i * ZW + BL],
                       M[:, i * W + N: i * W + N + BL],
                       ("dve", "act", "dve")[i])
            for r in range(1, 4):
                for i in range(NCH):
                    ps = pss.tile([P, BL], F32, tag="ap", bufs=3)
                    for c in range(NCH):
                        nc.tensor.matmul(
                            ps[:],
                            lhsT=mt_blk(c, i),
                            rhs=(M[:, c * W + N: c * W + N + BL] if r == 1
                                 else Z[:, c * ZW + (r - 1) * BL:
                                        c * ZW + r * BL]),
                            start=(c == 0), stop=(c == NCH - 1),
                        )
                    if r < 3:
                        e_copy(Z[:, i * ZW + r * BL: i * ZW + (r + 1) * BL],
                               ps[:], ("dve", "act", "dve")[i])
                    else:
                        z3ps.append(ps)   # z3 never needs SBUF

            # ---- stacked selection of all four branches; the branch pick
            # AND the log/mean loss tail run on host (it has the bits).
            # The Q|z1|z2 columns accumulate while z3 is still in flight;
            # z3's part multiplies straight out of PSUM.
            sel = pss.tile([1, ZW], F32, tag="ap", bufs=3)
            W3 = 3 * BL
            for c in range(NCH):
                tmp = tpool.tile([P, ZW], F32, tag="t2")
                e_tt(tmp[:, 0:W3], Z[:, c * ZW: c * ZW + W3],
                     OH4[:, c * ZW: c * ZW + W3], OP.mult,
                     ("dve", "pool", "dve")[c])
                nc.tensor.matmul(sel[:, 0:W3], lhsT=ONES[:],
                                 rhs=tmp[:, 0:W3], start=(c == 0),
                                 stop=(c == NCH - 1), skip_group_check=True)
            for c in range(NCH):
                t3 = tpool.tile([P, BL], F32, tag="t3")
                e_tt(t3[:], z3ps[c][:],
                     OH4[:, c * ZW + W3: (c + 1) * ZW], OP.mult, "dve")
                nc.tensor.matmul(sel[:, W3:ZW], lhsT=ONES[:], rhs=t3[:],
                                 start=(c == 0), stop=(c == NCH - 1),
                                 skip_group_check=True)
            OUT = tpool.tile([1, ZW], F32, tag="r0")
            nc.vector.tensor_copy(OUT[:], sel[:])
            nc.sync.dma_start(d_out[:], OUT[:])

    nc.compile()
    return nc


def _host_prep(c_mesh, gtheta, sigma_diff, init_color, delay_t, report_color):
    """Host-side glue: operator assembly (replicating reference f32 ops),
    plan selection, and per-core index/bit/layout arrays."""
    f32 = np.float32
    c = np.asarray(c_mesh, dtype=f32)
    g = np.asarray(gtheta, dtype=f32)
    s = np.asarray(sigma_diff, dtype=f32)[0]
    init = np.asarray(init_color, dtype=f32)
    t = np.asarray(delay_t, dtype=f32)
    rep = np.asarray(report_color, dtype=f32)

    d = (c[1] - c[0]).astype(f32)
    eye = np.eye(N, dtype=f32)
    up = np.roll(eye, -1, axis=1)
    dn = np.roll(eye, 1, axis=1)
    D1 = ((up - dn) / (f32(2.0) * d)).astype(f32)
    D2 = ((up - f32(2.0) * eye + dn) / (d * d)).astype(f32)
    A = ((s ** f32(2.0)) / f32(2.0) * D2 - D1 * g[None, :]).astype(f32)

    anorm = np.abs(A.astype(np.float64)).sum(axis=1).max()
    k_bits, deg_p, deg_r = plan = _plan(anorm)
    T0 = T_MAX / (1 << k_bits)
    X = (A * f32(T0)).astype(f32)

    m = np.floor(t.astype(np.float64) / T0).astype(np.int64)
    m = np.clip(m, 0, (1 << k_bits) - 1)
    r = (t.astype(np.float64) - m * T0) / T0  # in X = T0*A units
    bits = ((m[:, None] >> np.arange(k_bits)[None, :]) & 1)     # [B, K]
    idx = np.argmin(np.abs(c[None, :] - rep[:, None]), axis=1)

    # X|XT image, per-chunk interleaved: [XNc | XTc] so the first prelude
    # matmul only needs the first chunk DMA
    xe = np.empty((P, NCH * N), f32)
    for ci in range(NCH):
        xe[:, ci * N: (ci + 1) * N] = X[ci * P:(ci + 1) * P, :]
    ke_vals = list(range(1, deg_p - 1)) + [deg_p, (deg_p - 1) * deg_p]
    if 1 not in ke_vals:
        ke_vals = [1] + ke_vals
    ke = np.empty((P, len(ke_vals) * P), f32)
    for j, v in enumerate(ke_vals):
        ke[:, j * P: (j + 1) * P] = v * np.eye(P, dtype=f32)

    # CM3[p, c*BL+b] = c_mesh[c*P+p]
    cm3 = np.broadcast_to(
        c.reshape(NCH, P).T[:, :, None], (P, NCH, BL)).reshape(P, NCH * BL)

    shared = {
        "xe": xe,
        "ke": ke,
    }
    in_maps = []
    for core in range(NCORES):
        sl = slice(core * BL, (core + 1) * BL)
        irep = np.broadcast_to(np.tile(init[sl], NCH)[None, :],
                               (P, NCH * BL)).astype(f32)
        pv = np.concatenate([cm3, irep], axis=1).astype(f32)
        nmsk = max(k_bits - 2, 0)
        msk = np.broadcast_to(
            bits[sl, :nmsk].T.reshape(1, nmsk * BL), (P, nmsk * BL)
        ).astype(f32)  # bit j at [j*BL:(j+1)*BL]
        rdk = np.empty((deg_r, BL), f32)
        for k in range(1, deg_r + 1):
            rdk[k - 1] = (r[sl] / k).astype(f32)
        rdk = np.broadcast_to(
            rdk.reshape(1, deg_r * BL), (P, deg_r * BL)).astype(f32)
        oh = np.zeros((NCH, P, BL), f32)
        for b, ix in enumerate(idx[sl]):
            oh[ix // P, ix % P, b] = 1.0
        # per chunk, replicated 4x to select all branch-tree rows at once
        oh4 = np.ascontiguousarray(
            np.tile(oh, (1, 1, 4)).transpose(1, 0, 2).reshape(P, NCH * 4 * BL))
        aux = np.concatenate([rdk, msk, oh4], axis=1).astype(f32)
        in_maps.append(dict(shared, pv=pv, aux=aux))
    return plan, in_maps


def _get_nc(plan):
    if plan not in _COMPILED:
        _COMPILED[plan] = _build_bass(*plan)
    return _COMPILED[plan]


def kernel(**inputs):
    from concourse.bass_utils import run_bass_kernel_spmd

    plan, in_maps = _host_prep(
        inputs["c_mesh"], inputs["gtheta"], inputs["sigma_diff"],
        inputs["init_color"], inputs["delay_t"], inputs["report_color"],
    )
    nc = _get_nc(plan)
    res = run_bass_kernel_spmd(nc, in_maps, list(range(NCORES)))
    # branch r of the tree (M^r Q, r = top two time-bits) at [r*BL + b]
    sAB = np.stack(
        [np.asarray(res.results[k]["sel"]).reshape(4, BL)
         for k in range(NCORES)]
    )  # [NCORES, 4, BL]
    k_bits = plan[0]
    t = np.asarray(inputs["delay_t"], dtype=np.float32)
    T0 = T_MAX / (1 << k_bits)
    m = np.clip(np.floor(t.astype(np.float64) / T0).astype(np.int64),
                0, (1 << k_bits) - 1)
    v = ((m >> (k_bits - 2)) & 3).reshape(NCORES, BL)
    sel = np.take_along_axis(sAB, v[:, None, :], axis=1)[:, 0, :].reshape(-1)
    terms = np.log(np.maximum(sel.astype(np.float64), 0.0) + EPS)
    loss = -np.mean(terms)
    return np.asarray(loss, dtype=np.float32)


# revision 95
# speedup vs baseline: 1.0141x; 1.0122x over previous
"""Trainium2 Bass kernel for the circular drift-diffusion loss (batched expm).

Reference computes  loss = -mean_b log(relu(e_{idx_b}^T expm(t_b*A) p0_b) + eps)
with A a fixed 360x360 circular advection-diffusion operator, t_b in [0,1000),
p0_b a von Mises density, over a batch of 256.

Algorithm (per core; batch sharded 32/core over 8 cores):
  * Quantize t_b = m_b*T0 + r_b with T0 = 1000/2^K, m_b < 2^K.
  * Build the propagator chain M_j = expm(2^j*T0*A) by repeated squaring.
    The prelude evaluates the Taylor of expm(T0*A) in Horner form
    G_k = I + (X/k)G_{k+1}: the +k*I terms ride the PE as accumulate-
    matmuls against host-sent scaled identities and the 1/k scales fold
    into the psum->SBUF copies, so the prelude needs no elementwise adds.
  * ALL wide matmuls run in fp32r: 1 PE row/cycle (vs 4 for fp32) when
    the moving dim >= 256.  Its rounding noise through the chain is
    ~1e-3 relative on the density, i.e. ~1e-4 on the log-loss -- two
    orders inside the 2e-2 budget -- and the Taylor tolerances are
    relaxed to match (which also shortens the chain to K=7).
  * Bits 0..K-3 of m_b apply as masked batched matvecs merged into the
    squaring matmuls (32 extra moving columns); the blends are
    arithmetic (old + msk*(new-old)) since copy_predicated can't write
    fp32r.  The top TWO bits are blend-free: the kernel emits the branch
    tree Q, MQ, M^2Q, M^3Q (M = M_{K-2}), selects all four, and the host
    picks per sample -- blends between narrow applies were pure latency.
  * Residual: Q <- Taylor_DEG_R(r_b A) Q with per-sample r folded into
    host-precomputed coefficient tables; the narrow matvec steps hide
    inside the prelude rounds.
  * p0 built on device as one [P, NCH*BL]-wide op chain on a single
    engine (min-of-squares fold, Estrin cos poly, Exp activation);
    selection via one-hot + PE column-sum.  The branch pick and the
    log/mean loss tail run on host (O(B) glue).
Scheduling notes: dummy warm-up matmuls burn the PE's 2x-slow p-state
ramp inside the initial DMA shadow; loads are split across both HWDGE
queues and the gpsimd SWDGE ring because DMA transfers serialize per
queue; elementwise ops are hand-pinned to DVE/ACT/Pool so the copies
that gate each level land on an engine that is free at that moment.
"""

import math

import numpy as np

# ---------------- static problem constants (hardcoded per contract) ----------
N = 360            # color mesh size
P = 120            # partition chunk (N = 3*P)
NCH = 3            # chunks
B = 256            # total batch
NCORES = 8
BL = B // NCORES   # per-core batch
QW = NCH * BL      # width of a full Q block
T_MAX = 1000.0
KAPPA = 400.0      # 1/SIGMA_INIT^2
EPS = 1e-5
TWO_PI = 6.283185307179586
# ln(1/(2*pi*i0e(400)))  [i0e(400) = 0.019953356281939987]
LNC = 2.076480848703078
# cos(sqrt(u)) on u in [0, pi^2] (|delta| folded to [0,pi]), power basis c0..c8
COS_COEF = [1.00000000e+00, -5.00000000e-01, 4.16666666e-02, -1.38888885e-03,
            2.48015646e-05, -2.75566515e-07, 2.08651966e-09, -1.13535474e-11,
            4.13131734e-14]

_COMPILED = {}


def _taylor_deg(x, tol, lo):
    """Smallest d with x^(d+1)/(d+1)! < tol."""
    d = lo
    term = x ** (d + 1) / math.factorial(d + 1)
    while term > tol and d < 40:
        d += 1
        term *= x / (d + 1)
    return d


def _plan(anorm):
    """Choose (k_bits, deg_p, deg_r) from ||A||_inf.  The time grid is
    T0 = T_MAX/2^k_bits; every squaring level applies one bit of the
    quantized delay.  Tolerances sit just under the fp32r rounding noise
    (~1e-3 through the chain), which the 2e-2 rel-err budget dwarfs."""
    xa = T_MAX * float(anorm)
    if xa <= 0.0:
        return 2, 4, 3
    k0 = max(2, min(16, math.ceil(math.log2(max(xa / 0.9, 2.0)))))

    def degrees(k):
        x0 = xa / (1 << k)
        # tolerances are RELATIVE error on the propagated density; the log
        # in the loss divides that by |loss|~10, so a few percent is still
        # an order of magnitude inside the 2e-2 budget.  Prelude truncation
        # is amplified ~2^(k/2) through the squarings; the residual Taylor
        # is applied once (no amplification).
        tol_p = max(min(2.5e-2 / 2 ** (k / 2), 2e-3), 5e-8)
        return _taylor_deg(x0, tol_p, 2), _taylor_deg(x0, 1e-2, 2)

    # pick k by explicit cost minimization with measured per-stage costs
    # (chain level ~3.0us, prelude step ~1.8us, taylor step ~0.3us wall)
    best = None
    for k in range(max(2, k0 - 2), min(16, k0 + 2) + 1):
        dp, dr = degrees(k)
        cost = (k - 1) * 3.0 + (dp - 1) * 1.8 + dr * 0.3
        if best is None or cost < best[0]:
            best = (cost, k, dp, dr)
    _, k, deg_p, deg_r = best
    return k, deg_p, deg_r


def _build_bass(k_bits, deg_p, deg_r):
    """Construct the Bass program (SPMD; identical on all 8 cores)."""
    import concourse.tile as tile
    from concourse import bacc, mybir

    F32 = mybir.dt.float32
    F32R = mybir.dt.float32r
    AF = mybir.ActivationFunctionType
    OP = mybir.AluOpType

    nc = bacc.Bacc("TRN2", target_bir_lowering=False, debug=False)

    NMSK = max(k_bits - 2, 0)
    AUXW = (deg_r + NMSK + 4 * NCH) * BL

    d_xe = nc.dram_tensor("xe", [P, NCH * N], F32R,
                          kind="ExternalInput").ap()   # X row-chunks
    ke_vals = list(range(1, deg_p - 1)) + [deg_p, (deg_p - 1) * deg_p]
    if 1 not in ke_vals:
        ke_vals = [1] + ke_vals
    ke_idx = {v: i for i, v in enumerate(ke_vals)}
    d_ke = nc.dram_tensor("ke", [P, len(ke_vals) * P], F32R,
                          kind="ExternalInput").ap()   # v*I_P per slot
    d_pv = nc.dram_tensor("pv", [P, 2 * QW], F32,
                          kind="ExternalInput").ap()   # [CM3|IREP]
    d_aux = nc.dram_tensor("aux", [P, AUXW], F32,
                           kind="ExternalInput").ap()  # [RDK|MSK|OH]
    d_out = nc.dram_tensor("sel", [1, 4 * BL], F32,
                           kind="ExternalOutput").ap()

    with tile.TileContext(nc) as tc:
        with (
            tc.tile_pool(name="const", bufs=1) as cpool,
            tc.tile_pool(name="mats", bufs=4) as mpool,
            tc.tile_pool(name="qp", bufs=2) as qpool,
            tc.tile_pool(name="vp", bufs=3) as vpool,
            tc.tile_pool(name="tp", bufs=4) as tpool,
            tc.tile_pool(name="psb", bufs=5, space="PSUM") as psb,
            tc.tile_pool(name="pss", bufs=3, space="PSUM") as pss,
        ):
            # ---- engine helpers: explicit pinning (GPSIMD can't read PSUM,
            # ACT can't do tensor_tensor; criticial-path copies go to the
            # engine that is free at that point of each level)
            def e_copy(dst, src, eng, scale=None):
                if scale is not None:
                    if eng == "dve":
                        nc.vector.tensor_scalar(dst, src, scale, None,
                                                op0=OP.mult)
                    elif eng == "act":
                        nc.scalar.mul(dst, src, scale)
                    else:
                        nc.gpsimd.tensor_scalar(dst, src, scale, None,
                                                op0=OP.mult)
                else:
                    if eng == "dve":
                        nc.vector.tensor_copy(dst, src)
                    elif eng == "act":
                        nc.scalar.copy(dst, src)
                    else:
                        nc.gpsimd.tensor_copy(dst, src)

            def e_tt(dst, a, b, op, eng):
                (nc.vector if eng == "dve" else nc.gpsimd).tensor_tensor(
                    dst, a, b, op=op)

            # ---- constants.  HWDGE costs ~625ns of serialized ring per
            # DMA, so the critical loads (pv for p0, then the X|XT chunks)
            # go there in need-order while everything else rides the
            # separate software-DGE ring (gpsimd-issued).
            # transfers serialize per hardware queue, so spread the loads
            # over all three HWDGE queues (SP/DVE/ACT): the X|XT chunks land
            # concurrently ~4.4us in instead of staggering 1us apart
            # X^T is built ON DEVICE (PE transposes in the warmup shadow):
            # only X rides the serialized DMA transfer FIFO.  The identity
            # (ke slot 0) goes first -- the transposes need it.
            PV = cpool.tile([P, 2 * QW], F32, tag="pv")
            CXT = cpool.tile([P, NCH * N], F32R, tag="cxt")
            KE = cpool.tile([P, len(ke_vals) * P], F32R, tag="ke")
            nc.sync.dma_start(KE[:, 0:P], d_ke[:, 0:P])
            nc.sync.dma_start(CXT[:, 0:N], d_xe[:, 0:N])
            nc.scalar.dma_start(CXT[:, N: 2 * N], d_xe[:, N: 2 * N])
            nc.gpsimd.dma_start(PV[:], d_pv[:])
            nc.sync.dma_start(CXT[:, 2 * N: 3 * N], d_xe[:, 2 * N: 3 * N])
            if len(ke_vals) > 1:
                nc.gpsimd.dma_start(KE[:, P:], d_ke[:, P:])
            AUX = cpool.tile([P, AUXW], F32, tag="aux")
            nc.gpsimd.dma_start(AUX[:], d_aux[:])

            def ke_blk(v):
                return KE[:, ke_idx[v] * P: (ke_idx[v] + 1) * P]
            # fp32r identity for transposes (bf16 would rate 1.0 cyc/row vs
            # 1.5 but lowers to Ldweights+matmul pairs that stall the PE)
            E120 = KE[:, 0:P]   # ke slot 0 is 1*I

            # warm-up matmuls: the PE runs its first ~3us at the mid p-state
            # (2x cycle time); burning that ramp on dummies while the DMAs
            # land makes the real prelude run at full clock
            W0 = cpool.tile([P, N], F32, tag="w0m")
            nc.vector.memset(W0[:], 0.0)
            for wmw in (N, 250):
                wps = psb.tile([P, N], F32, tag="sq", bufs=3)
                nc.tensor.matmul(wps[:, 0:wmw], lhsT=W0[:, 0:P],
                                 rhs=W0[:, 0:wmw], start=True, stop=True)

            XTD = cpool.tile([P, NCH * N], F32R, tag="xtd")

            def xn(c):
                return CXT[:, c * N: (c + 1) * N]

            def xt_blk(c, i):
                return XTD[:, c * N + i * P: c * N + (i + 1) * P]

            RDK = AUX[:, 0: deg_r * BL]
            MSK = AUX[:, deg_r * BL: (deg_r + NMSK) * BL]
            OH4 = AUX[:, (deg_r + NMSK) * BL: AUXW]

            ONES = cpool.tile([P, 1], F32, tag="ones")
            nc.vector.memset(ONES[:], 1.0)
            BEXP = cpool.tile([P, 1], F32, tag="bexp")
            nc.vector.memset(BEXP[:], LNC - KAPPA)

            W = N + BL  # merged chunk width: [M_c | Q_c]

            def mm_group(ps, lhsT_of, rhs_tile, i, rhs_w, rhs_stride=None):
                rs = rhs_w if rhs_stride is None else rhs_stride
                for c in range(NCH):
                    nc.tensor.matmul(
                        ps[:],
                        lhsT=lhsT_of(c, i),
                        rhs=rhs_tile[:, c * rs: c * rs + rhs_w],
                        start=(c == 0), stop=(c == NCH - 1),
                    )

            # ---- p0 (von Mises) as one [P, QW]-wide op chain.  Latency
            # matters (p0 gates the residual Taylor steps hidden inside the
            # prelude), so: u = min(d^2, (d-2pi)^2, (d+2pi)^2) needs no Abs
            # round-trip, and cos(sqrt(u)) evaluates Estrin-style (depth ~7)
            # split across DVE and Pool.
            CM3 = PV[:, 0:QW]
            IREP = PV[:, QW:2 * QW]
            Q = qpool.tile([P, QW], F32R, tag="q")
            # one engine (DVE) end to end: no cross-engine semaphore hops on
            # the latency spine, and DVE's prelude work (fused S-updates)
            # can lag since S is only read at the chain's first level
            V = nc.vector
            dl = tpool.tile([P, QW], F32, tag="w0")
            V.tensor_tensor(dl[:], IREP, CM3, op=OP.subtract)
            bm = tpool.tile([P, QW], F32, tag="w1")
            V.tensor_scalar(bm[:], dl[:], 1.0, -TWO_PI,
                            op0=OP.mult, op1=OP.add)
            cp_ = tpool.tile([P, QW], F32, tag="w2")
            V.tensor_scalar(cp_[:], dl[:], 1.0, TWO_PI,
                            op0=OP.mult, op1=OP.add)
            a2 = tpool.tile([P, QW], F32, tag="w3")
            V.tensor_tensor(a2[:], dl[:], dl[:], op=OP.mult)
            b2 = tpool.tile([P, QW], F32, tag="w1")
            V.tensor_tensor(b2[:], bm[:], bm[:], op=OP.mult)
            c2 = tpool.tile([P, QW], F32, tag="w2")
            V.tensor_tensor(c2[:], cp_[:], cp_[:], op=OP.mult)
            u = tpool.tile([P, QW], F32, tag="w0")
            V.tensor_tensor(u[:], b2[:], c2[:], op=OP.min)
            V.tensor_tensor(u[:], u[:], a2[:], op=OP.min)
            # Estrin: pairs via fused (x*s0 + s1), then combine with powers
            u2 = tpool.tile([P, QW], F32, tag="w1")
            V.tensor_tensor(u2[:], u[:], u[:], op=OP.mult)
            p01 = tpool.tile([P, QW], F32, tag="w2")
            V.tensor_scalar(p01[:], u[:], COS_COEF[1], COS_COEF[0],
                            op0=OP.mult, op1=OP.add)
            p23 = tpool.tile([P, QW], F32, tag="w3")
            V.tensor_scalar(p23[:], u[:], COS_COEF[3], COS_COEF[2],
                            op0=OP.mult, op1=OP.add)
            p45 = tpool.tile([P, QW], F32, tag="w4")
            V.tensor_scalar(p45[:], u[:], COS_COEF[5], COS_COEF[4],
                            op0=OP.mult, op1=OP.add)
            p67 = tpool.tile([P, QW], F32, tag="w5")
            V.tensor_scalar(p67[:], u[:], COS_COEF[7], COS_COEF[6],
                            op0=OP.mult, op1=OP.add)
            u4 = tpool.tile([P, QW], F32, tag="w6")
            V.tensor_tensor(u4[:], u2[:], u2[:], op=OP.mult)
            # q67 = p67 + c8*u2 ; hi = p45 + q67*u2 ; lo = p01 + p23*u2
            q67 = tpool.tile([P, QW], F32, tag="w1")
            V.tensor_scalar(q67[:], u2[:], COS_COEF[8], None, op0=OP.mult)
            V.tensor_tensor(q67[:], q67[:], p67[:], op=OP.add)
            V.tensor_tensor(q67[:], q67[:], u2[:], op=OP.mult)
            hi = tpool.tile([P, QW], F32, tag="w4")
            V.tensor_tensor(hi[:], p45[:], q67[:], op=OP.add)
            lo = tpool.tile([P, QW], F32, tag="w2")
            V.tensor_tensor(p23[:], p23[:], u2[:], op=OP.mult)
            V.tensor_tensor(lo[:], p01[:], p23[:], op=OP.add)
            h = tpool.tile([P, QW], F32, tag="w0")
            V.tensor_tensor(h[:], hi[:], u4[:], op=OP.mult)
            V.tensor_tensor(h[:], h[:], lo[:], op=OP.add)
            # p0 = exp(kappa*cos - kappa + lnC)
            nc.scalar.activation(Q[:], h[:], AF.Exp, bias=BEXP[:],
                                 scale=KAPPA)

            # ---- residual Taylor on p0 (commutes with the bit applies):
            # V = Q + rdk_k*(X V), k=deg_r..1, as narrow matvec groups that
            # hide inside the prelude (one per round from the second round
            # on; leftovers drain under the S transpose).  The k==1 step
            # lands the evolved p0 straight in the MQ tile's Q slots.
            S = mpool.tile([P, NCH * W], F32R, tag="M")
            tay = {"V": (Q, BL, 0), "k": deg_r}

            def taylor_step():
                if tay["k"] < 1:
                    return
                k = tay["k"]
                Vt, vstr, voff = tay["V"]
                Vn = None if k == 1 else vpool.tile([P, QW], F32R, tag="V")
                for i in range(NCH):
                    ps = pss.tile([P, BL], F32, tag="ap", bufs=3)
                    for c in range(NCH):
                        nc.tensor.matmul(
                            ps[:], lhsT=xt_blk(c, i),
                            rhs=Vt[:, c * vstr + voff: c * vstr + voff + BL],
                            start=(c == 0), stop=(c == NCH - 1))
                    vs = (S[:, i * W + N: (i + 1) * W] if k == 1
                          else Vn[:, i * BL:(i + 1) * BL])
                    e_tt(vs, ps[:], RDK[:, (k - 1) * BL: k * BL], OP.mult,
                         "dve")
                    e_tt(vs, vs, Q[:, i * BL:(i + 1) * BL], OP.add, "pool")
                tay["V"] = (Vn, BL, 0)
                tay["k"] = k - 1

            # ---- prelude: Horner form of the Taylor S = sum X^j/j! --------
            # G_k = I + (X/k) G_{k+1}, k = deg_p-1..1; S = G_1.  The +k*I
            # rides the PE as one extra accumulate-matmul per chunk (host
            # sends k-scaled identities) and the 1/k folds into the psum
            # copy, so the prelude has NO elementwise adds at all.  The
            # final round's copy lands G_1 straight in the MQ tile's S
            # slots; M tiles are MQ-shaped ([M_c | Q_c] per chunk) so the
            # bit-applies merge into the squaring matmuls.
            # the first round folds G_{deg_p} in: psum = X@X + deg_p*X +
            # (deg_p-1)*deg_p*I, scaled by 1/((deg_p-1)*deg_p), which equals
            # G_{deg_p-1} -- so no G init chain gates the first matmuls
            def build_xt(i):
                # XTD block (c,i) = transpose of X chunk i's column-block c
                pst = psb.tile([P, N], F32R, tag="tr", bufs=2)
                for c in range(NCH):
                    nc.tensor.transpose(
                        pst[:, c * P:(c + 1) * P],
                        CXT[:, i * N + c * P: i * N + (c + 1) * P],
                        E120[:])
                dst = XTD[:].rearrange(
                    "p (c n) -> p c n", c=NCH)[:, :, i * P:(i + 1) * P]
                e_copy(dst, pst[:].rearrange("p (c n) -> p c n", c=NCH),
                       ("dve", "act", "dve")[i])

            build_xt(0)
            build_xt(1)
            build_xt(2)

            G = None
            for ridx, k in enumerate(range(deg_p - 1, 0, -1)):
                first = G is None
                lastr = k == 1
                Gn = (S if lastr
                      else mpool.tile([P, NCH * N], F32R, tag="T"))
                for i in range(NCH):
                    ps = psb.tile([P, N], F32, tag="sq", bufs=3)
                    for c in range(NCH):
                        nc.tensor.matmul(
                            ps[:], lhsT=xt_blk(c, i),
                            rhs=(xn(c) if first
                                 else G[:, c * N: (c + 1) * N]),
                            start=(c == 0), stop=False)
                    # +v*I rides as a [P,P] sub-range accumulate: the
                    # identity slab is zero outside column-block i
                    if first:
                        nc.tensor.matmul(
                            ps[:], lhsT=ke_blk(deg_p), rhs=xn(i),
                            start=False, stop=False)
                        nc.tensor.matmul(
                            ps[:, i * P: (i + 1) * P],
                            lhsT=ke_blk((deg_p - 1) * deg_p), rhs=E120,
                            start=False, stop=True, skip_group_check=True)
                        scale = 1.0 / ((deg_p - 1) * deg_p)
                    else:
                        nc.tensor.matmul(
                            ps[:, i * P: (i + 1) * P], lhsT=ke_blk(k),
                            rhs=E120,
                            start=False, stop=True, skip_group_check=True)
                        scale = None if k == 1 else 1.0 / k
                    dst = (Gn[:, i * W: i * W + N] if lastr
                           else Gn[:, i * N: (i + 1) * N])
                    e_copy(dst, ps[:], "act", scale=scale)
                G = Gn
                if ridx >= 1:
                    # p0 is ready by the end of round two; the PE is in-order
                    # so a step emitted earlier would stall the round matmuls
                    taylor_step()

            ST = mpool.tile([P, NCH * N], F32R, tag="MT")

            def transpose_mq(MTt, Mt):
                # phase ib: transpose the 3 column-blocks of S chunk ib into
                # one [P, N] psum, then a single strided copy into MT's
                # column-block ib of every chunk.  Phase 0 lands first so the
                # next level's first matmul group unblocks early.
                for ib in range(NCH):
                    pst = psb.tile([P, N], F32R, tag="tr", bufs=2)
                    for cp in range(NCH):
                        nc.tensor.transpose(
                            pst[:, cp * P:(cp + 1) * P],
                            Mt[:, ib * W + cp * P: ib * W + cp * P + P],
                            E120[:],
                        )
                    dst = MTt[:].rearrange(
                        "p (c n) -> p c n", c=NCH)[:, :, ib * P:(ib + 1) * P]
                    pv3 = pst[:].rearrange("p (c n) -> p c n", c=NCH)
                    e_copy(dst, pv3, ("act", "act", "dve")[ib])

            transpose_mq(ST, S)
            M, MT = S, ST

            def mt_blk(c, i):
                return MT[:, c * N + i * P: c * N + (i + 1) * P]

            def blend_q(dst, old, new_ps, bit):
                # dst = old + msk*(new - old); copy_predicated can't write
                # fp32r-typed tiles, so blend arithmetically (msk is 0/1)
                t = tpool.tile([P, BL], F32, tag="t3")
                e_tt(t[:], new_ps, old, OP.subtract, "dve")
                e_tt(t[:], t[:], MSK[:, bit * BL:(bit + 1) * BL], OP.mult,
                     "pool")
                e_tt(dst, old, t[:], OP.add, "pool")

            def square(Mc, MTc, MTc_blk, bit=None):
                # Sn = Mc@Mc; if bit is not None also compute Mc@Q (merged
                # columns) and blend it into Sn's Q slot under the bit mask.
                Sn = mpool.tile([P, NCH * W], F32R, tag="M")
                STn = mpool.tile([P, NCH * N], F32R, tag="MT")
                wid = N if bit is None else W
                for i in range(NCH):
                    ps = psb.tile([P, wid], F32, tag="sq", bufs=3)
                    mm_group(ps, MTc_blk, Mc, i, wid, rhs_stride=W)
                    e_copy(Sn[:, i * W: i * W + N], ps[:, :N],
                           ("act", "dve", "dve")[i])
                    if bit is not None:
                        blend_q(Sn[:, i * W + N: (i + 1) * W],
                                Mc[:, i * W + N: (i + 1) * W],
                                ps[:, N:W], bit)
                transpose_mq(STn, Sn)
                return Sn, STn

            # drain remaining taylor steps; the k==1 step lands the
            # evolved p0 directly in the MQ tile's Q slots
            while tay["k"] >= 1:
                taylor_step()

            # ---- merged bit applies + chain squarings ---------------------
            # level j squares M (= expm(2^j T0 A)) and applies bit j of the
            # quantized delay to Q in the same matmul set.  The top TWO bits
            # need no further squaring: bit k-2 is a single apply of M_{k-2}
            # and bit k-1 a double apply (M_{k-1} Q = M_{k-2} (M_{k-2} Q)),
            # which is ~2x cheaper than materializing M_{k-1}.
            for j in range(k_bits - 2):
                M, MT = square(M, MT, mt_blk, bit=j)

            # Z: per chunk [Q_c | z1_c | z2_c | z3_c], z_r = M^r Q with
            # M = M_{k-2}: a blend-free branch tree over the top two bits
            # (masked blends between narrow applies were pure semaphore
            # latency); the host picks the branch per sample.
            ZW = 4 * BL
            Z = qpool.tile([P, NCH * ZW], F32R, tag="z")
            z3ps = []
            for i in range(NCH):
                e_copy(Z[:, i * ZW: i * ZW + BL],
                       M[:, i * W + N: i * W + N + BL],
                       ("dve", "act", "dve")[i])
            for r in range(1, 4):
                for i in range(NCH):
                    ps = pss.tile([P, BL], F32, tag="ap", bufs=3)
                    for c in range(NCH):
                        nc.tensor.matmul(
                            ps[:],
                            lhsT=mt_blk(c, i),
                            rhs=(M[:, c * W + N: c * W + N + BL] if r == 1
                                 else Z[:, c * ZW + (r - 1) * BL:
                                        c * ZW + r * BL]),
                            start=(c == 0), stop=(c == NCH - 1),
                        )
                    if r < 3:
                        e_copy(Z[:, i * ZW + r * BL: i * ZW + (r + 1) * BL],
                               ps[:], ("dve", "act", "dve")[i])
                    else:
                        z3ps.append(ps)   # z3 never needs SBUF

            # ---- stacked selection of all four branches; the branch pick
            # AND the log/mean loss tail run on host (it has the bits).
            # The Q|z1|z2 columns accumulate while z3 is still in flight;
            # z3's part multiplies straight out of PSUM.
            sel = pss.tile([1, ZW], F32, tag="ap", bufs=3)
            W3 = 3 * BL
            for c in range(NCH):
                tmp = tpool.tile([P, ZW], F32, tag="t2")
                e_tt(tmp[:, 0:W3], Z[:, c * ZW: c * ZW + W3],
                     OH4[:, c * ZW: c * ZW + W3], OP.mult,
                     ("dve", "pool", "dve")[c])
                nc.tensor.matmul(sel[:, 0:W3], lhsT=ONES[:],
                                 rhs=tmp[:, 0:W3], start=(c == 0),
                                 stop=(c == NCH - 1), skip_group_check=True)
            for c in range(NCH):
                t3 = tpool.tile([P, BL], F32, tag="t3")
                e_tt(t3[:], z3ps[c][:],
                     OH4[:, c * ZW + W3: (c + 1) * ZW], OP.mult, "dve")
                nc.tensor.matmul(sel[:, W3:ZW], lhsT=ONES[:], rhs=t3[:],
                                 start=(c == 0), stop=(c == NCH - 1),
                                 skip_group_check=True)
            OUT = tpool.tile([1, ZW], F32, tag="r0")
            nc.vector.tensor_copy(OUT[:], sel[:])
            nc.sync.dma_start(d_out[:], OUT[:])

    nc.compile()
    return nc


def _host_prep(c_mesh, gtheta, sigma_diff, init_color, delay_t, report_color):
    """Host-side glue: operator assembly (replicating reference f32 ops),
    plan selection, and per-core index/bit/layout arrays."""
    f32 = np.float32
    c = np.asarray(c_mesh, dtype=f32)
    g = np.asarray(gtheta, dtype=f32)
    s = np.asarray(sigma_diff, dtype=f32)[0]
    init = np.asarray(init_color, dtype=f32)
    t = np.asarray(delay_t, dtype=f32)
    rep = np.asarray(report_color, dtype=f32)

    d = (c[1] - c[0]).astype(f32)
    eye = np.eye(N, dtype=f32)
    up = np.roll(eye, -1, axis=1)
    dn = np.roll(eye, 1, axis=1)
    D1 = ((up - dn) / (f32(2.0) * d)).astype(f32)
    D2 = ((up - f32(2.0) * eye + dn) / (d * d)).astype(f32)
    A = ((s ** f32(2.0)) / f32(2.0) * D2 - D1 * g[None, :]).astype(f32)

    anorm = np.abs(A.astype(np.float64)).sum(axis=1).max()
    k_bits, deg_p, deg_r = plan = _plan(anorm)
    T0 = T_MAX / (1 << k_bits)
    X = (A * f32(T0)).astype(f32)

    m = np.floor(t.astype(np.float64) / T0).astype(np.int64)
    m = np.clip(m, 0, (1 << k_bits) - 1)
    r = (t.astype(np.float64) - m * T0) / T0  # in X = T0*A units
    bits = ((m[:, None] >> np.arange(k_bits)[None, :]) & 1)     # [B, K]
    idx = np.argmin(np.abs(c[None, :] - rep[:, None]), axis=1)

    # X|XT image, per-chunk interleaved: [XNc | XTc] so the first prelude
    # matmul only needs the first chunk DMA
    xe = np.empty((P, NCH * N), f32)
    for ci in range(NCH):
        xe[:, ci * N: (ci + 1) * N] = X[ci * P:(ci + 1) * P, :]
    ke_vals = list(range(1, deg_p - 1)) + [deg_p, (deg_p - 1) * deg_p]
    if 1 not in ke_vals:
        ke_vals = [1] + ke_vals
    ke = np.empty((P, len(ke_vals) * P), f32)
    for j, v in enumerate(ke_vals):
        ke[:, j * P: (j + 1) * P] = v * np.eye(P, dtype=f32)

    # CM3[p, c*BL+b] = c_mesh[c*P+p]
    cm3 = np.broadcast_to(
        c.reshape(NCH, P).T[:, :, None], (P, NCH, BL)).reshape(P, NCH * BL)

    shared = {
        "xe": xe,
        "ke": ke,
    }
    in_maps = []
    for core in range(NCORES):
        sl = slice(core * BL, (core + 1) * BL)
        irep = np.broadcast_to(np.tile(init[sl], NCH)[None, :],
                               (P, NCH * BL)).astype(f32)
        pv = np.concatenate([cm3, irep], axis=1).astype(f32)
        nmsk = max(k_bits - 2, 0)
        msk = np.broadcast_to(
            bits[sl, :nmsk].T.reshape(1, nmsk * BL), (P, nmsk * BL)
        ).astype(f32)  # bit j at [j*BL:(j+1)*BL]
        rdk = np.empty((deg_r, BL), f32)
        for k in range(1, deg_r + 1):
            rdk[k - 1] = (r[sl] / k).astype(f32)
        rdk = np.broadcast_to(
            rdk.reshape(1, deg_r * BL), (P, deg_r * BL)).astype(f32)
        oh = np.zeros((NCH, P, BL), f32)
        for b, ix in enumerate(idx[sl]):
            oh[ix // P, ix % P, b] = 1.0
        # per chunk, replicated 4x to select all branch-tree rows at once
        oh4 = np.ascontiguousarray(
            np.tile(oh, (1, 1, 4)).transpose(1, 0, 2).reshape(P, NCH * 4 * BL))
        aux = np.concatenate([rdk, msk, oh4], axis=1).astype(f32)
        in_maps.append(dict(shared, pv=pv, aux=aux))
    return plan, in_maps


def _get_nc(plan):
    if plan not in _COMPILED:
        _COMPILED[plan] = _build_bass(*plan)
    return _COMPILED[plan]


def kernel(**inputs):
    from concourse.bass_utils import run_bass_kernel_spmd

    plan, in_maps = _host_prep(
        inputs["c_mesh"], inputs["gtheta"], inputs["sigma_diff"],
        inputs["init_color"], inputs["delay_t"], inputs["report_color"],
    )
    nc = _get_nc(plan)
    res = run_bass_kernel_spmd(nc, in_maps, list(range(NCORES)))
    # branch r of the tree (M^r Q, r = top two time-bits) at [r*BL + b]
    sAB = np.stack(
        [np.asarray(res.results[k]["sel"]).reshape(4, BL)
         for k in range(NCORES)]
    )  # [NCORES, 4, BL]
    k_bits = plan[0]
    t = np.asarray(inputs["delay_t"], dtype=np.float32)
    T0 = T_MAX / (1 << k_bits)
    m = np.clip(np.floor(t.astype(np.float64) / T0).astype(np.int64),
                0, (1 << k_bits) - 1)
    v = ((m >> (k_bits - 2)) & 3).reshape(NCORES, BL)
    sel = np.take_along_axis(sAB, v[:, None, :], axis=1)[:, 0, :].reshape(-1)
    terms = np.log(np.maximum(sel.astype(np.float64), 0.0) + EPS)
    loss = -np.mean(terms)
    return np.asarray(loss, dtype=np.float32)
